# revision 14
# baseline (speedup 1.0000x reference)
"""GAT layer kernel for Trainium2, sharded across 8 NeuronCores.

Strategy (edge partitioning, per sharding hint):
  Stage 1 (node-sharded SPMD): per-head projection Wh = h @ W + bW fused with
    the attention partials s_src = Wh.a_src, s_dst = Wh.a_dst + ba via one
    augmented matmul per 128-node chunk -> fp16 table
    T[n] = [Wh (128) | s_src (4) | s_dst (4) | pad] (512B rows).
  Host: concat T slices, build the per-core edge-block schedule.
  Stage 2 (edge-sharded SPMD): edges grouped by destination into blocks
    (<=128 dst nodes, 16 chunks x 128 edge slots; chunks 0-7 hold edges with
    src < 25088, chunks 8-15 the rest, so gather indices fit in int16).
    Per block: two dma_gather ops pull T[src] rows, alpha =
    exp(leakyrelu(s_src + s_dst)) is computed on DVE/ACT, gathered Wh rows are
    scaled by alpha, and per-chunk one-hot matmuls scatter-add [num | denom]
    into a [dst_node, 132] PSUM accumulator.  Softmax max-subtraction is
    skipped (|e| = O(1) and softmax is shift-invariant).
    Finalize: out = num * (1/denom).
"""

import numpy as np
from contextlib import ExitStack

from concourse import bass, bacc, tile
import concourse.mybir as mybir
from concourse.bass_utils import run_bass_kernel_spmd

F32 = mybir.dt.float32
F16 = mybir.dt.float16
I16 = mybir.dt.int16

# Problem constants (nn_GAT_36009005809883)
N, E, F_IN, H, D = 50000, 800000, 256, 4, 32
ALPHA = 0.2
NCORES = 8
FOUT = H * D             # 128
TCOLS = FOUT + 2 * H     # 136 = Wh | s_src | s_dst
GCOLS = FOUT + H         # 132 used per edge: Wh | s_src
TROW = 256               # fp16 table row (512B)
NPAD = 50176             # 392 chunks of 128 nodes, 49 per core
NSLICE = NPAD // NCORES  # 6272
LO_N = 25088             # table split: both halves' indices fit int16
C_LO, C_HI = 8, 8        # dma_gather num_idxs is capped at 1024
SLOT_I = C_LO + C_HI     # 17 chunks per block
CAP_LO, CAP_HI = C_LO * 128, C_HI * 128
DUMMY_LOC = 300.0

_cache = {}
ACT_EXPAND = True


# ----------------------------------------------------------------- stage 1
def build_stage1(repeat=1):
    nc = bacc.Bacc("TRN2", target_bir_lowering=False, debug=False,
                   num_devices=NCORES)
    hT = nc.dram_tensor("hT", [F_IN, NSLICE], F16, kind="ExternalInput").ap()
    waug = nc.dram_tensor("waug", [F_IN, TCOLS], F16, kind="ExternalInput").ap()
    biasb = nc.dram_tensor("biasb", [128, TCOLS], F32, kind="ExternalInput").ap()
    tout = nc.dram_tensor("tout", [NSLICE, TROW], F16, kind="ExternalOutput").ap()

    nchunks = NSLICE // 128
    with tile.TileContext(nc) as tc:
        with ExitStack() as ctx:
            consts = ctx.enter_context(tc.tile_pool(name="consts", bufs=1))
            lhs_pool = ctx.enter_context(tc.tile_pool(name="lhs", bufs=6))
            out_pool = ctx.enter_context(tc.tile_pool(name="out", bufs=3))
            psum_pool = ctx.enter_context(
                tc.tile_pool(name="ps", bufs=2, space="PSUM"))

            w_t = consts.tile([128, 2, TCOLS], F16)
            nc.sync.dma_start(w_t[:], waug.rearrange("(k p) f -> p k f", p=128))
            b_t = consts.tile([128, TCOLS], F32)
            nc.sync.dma_start(b_t[:], biasb[:])

            for _r in range(repeat):
              for c in range(nchunks):
                ps = psum_pool.tile([128, TCOLS], F32)
                for kt in range(2):
                    lt = lhs_pool.tile([128, 128], F16, tag="lhs")
                    nc.sync.dma_start(
                        lt[:], hT[kt * 128:(kt + 1) * 128,
                                  c * 128:(c + 1) * 128])
                    nc.tensor.matmul(ps[:], lt[:], w_t[:, kt, :],
                                     start=(kt == 0), stop=(kt == 1))
                ot = out_pool.tile([128, TCOLS], F16)
                nc.vector.tensor_add(ot[:], ps[:], b_t[:])
                nc.sync.dma_start(tout[c * 128:(c + 1) * 128, 0:TCOLS], ot[:])
    nc.compile()
    return nc


# ----------------------------------------------------------------- stage 2
def build_stage2(B, n_lo, n_hi, repeat=1, act_expand=None):
    global ACT_EXPAND
    if act_expand is not None:
        ACT_EXPAND = act_expand
    nc = bacc.Bacc("TRN2", target_bir_lowering=False, debug=False,
                   num_devices=NCORES)
    tfull = nc.dram_tensor("tfull", [NPAD, TROW], F16, kind="ExternalInput").ap()
    lowi = nc.dram_tensor("lowi", [B, 128, CAP_LO // 16], I16,
                          kind="ExternalInput").ap()
    hiwi = nc.dram_tensor("hiwi", [B, 128, CAP_HI // 16], I16,
                          kind="ExternalInput").ap()
    dloc = nc.dram_tensor("dloc", [B, 128, SLOT_I], F32, kind="ExternalInput").ap()
    sdst = nc.dram_tensor("sdst", [B, 128, SLOT_I, H], F16,
                          kind="ExternalInput").ap()
    iota = nc.dram_tensor("iota", [128, 128], F16, kind="ExternalInput").ap()
    outs = nc.dram_tensor("outs", [B * 128, FOUT], F16, kind="ExternalOutput").ap()

    tlo = tfull[0:LO_N, :]
    thi = tfull[LO_N:NPAD, :]

    with tile.TileContext(nc) as tc:
        with ExitStack() as ctx:
            consts = ctx.enter_context(tc.tile_pool(name="consts", bufs=1))
            idx_pool = ctx.enter_context(tc.tile_pool(name="idx", bufs=3))
            val_pool = ctx.enter_context(tc.tile_pool(name="vals", bufs=3))
            wv_pool = ctx.enter_context(tc.tile_pool(name="wv", bufs=3))
            sm_pool = ctx.enter_context(tc.tile_pool(name="small", bufs=3))
            m_pool = ctx.enter_context(tc.tile_pool(name="oneh", bufs=2 * SLOT_I))
            fin_pool = ctx.enter_context(tc.tile_pool(name="fin", bufs=3))
            psum_pool = ctx.enter_context(
                tc.tile_pool(name="ps", bufs=2, space="PSUM"))

            iota_t = consts.tile([128, 128], F16)
            nc.sync.dma_start(iota_t[:], iota[:])

            # zero-init every vals buffer once: per-core -1-trimmed gathers
            # leave trailing slots unwritten, and SBUF garbage at kernel
            # start could be NaN (NaN * 0 would poison the PSUM accum).
            for _i in range(3):
                vz = val_pool.tile([128, SLOT_I, TROW], F16)
                nc.vector.memset(vz[:], 0.0)

            for _r in range(repeat):
              for b in range(B):
                # active chunks (counts are pre-rounded to multiples of 128,
                # so every active chunk is fully gathered -- compute never
                # touches uninitialized SBUF, which ACT would trap on)
                a_lo = int(n_lo[b]) // 128
                a_hi = int(n_hi[b]) // 128
                halves = []
                if a_lo:
                    halves.append((0, a_lo))
                if a_hi:
                    halves.append((C_LO, C_LO + a_hi))
                if not halves:
                    zt = fin_pool.tile([128, FOUT], F16, tag="ot")
                    nc.vector.memset(zt[:], 0.0)
                    nc.sync.dma_start(outs[b * 128:(b + 1) * 128, :], zt[:])
                    continue

                lct = idx_pool.tile([128, SLOT_I], F32, tag="lct")
                nc.sync.dma_start(lct[:], dloc[b])
                sdt = idx_pool.tile([128, SLOT_I, H], F16, tag="sdt")
                nc.sync.dma_start(sdt[:], sdst[b])

                vals = val_pool.tile([128, SLOT_I, TROW], F16)
                if a_lo:
                    lit = idx_pool.tile([128, CAP_LO // 16], I16, tag="lit")
                    nc.sync.dma_start(lit[:], lowi[b])
                    nc.gpsimd.dma_gather(vals[:, 0:C_LO, :], tlo, lit[:],
                                         CAP_LO, int(n_lo[b]), TROW)
                if a_hi:
                    hit = idx_pool.tile([128, CAP_HI // 16], I16, tag="hit")
                    nc.sync.dma_start(hit[:], hiwi[b])
                    nc.gpsimd.dma_gather(vals[:, C_LO:SLOT_I, :], thi, hit[:],
                                         CAP_HI, int(n_hi[b]), TROW)

                # e = s_src + s_dst ; leaky ; exp ; weighted values
                # (restricted to active chunks per half)
                ev = sm_pool.tile([128, SLOT_I, H], F32, tag="ev")
                t1 = sm_pool.tile([128, SLOT_I, H], F32, tag="t1")
                el = sm_pool.tile([128, SLOT_I, H], F32, tag="el")
                exe = sm_pool.tile([128, SLOT_I, FOUT], F16, tag="exe")
                wv = wv_pool.tile([128, SLOT_I, GCOLS], F16)
                for (c0, c1) in halves:
                    w = c1 - c0
                    nc.vector.tensor_add(ev[:, c0:c1, :],
                                         vals[:, c0:c1, FOUT:GCOLS],
                                         sdt[:, c0:c1, :])
                    nc.vector.tensor_scalar_mul(t1[:, c0:c1, :],
                                                ev[:, c0:c1, :], ALPHA)
                    nc.vector.tensor_max(el[:, c0:c1, :], ev[:, c0:c1, :],
                                         t1[:, c0:c1, :])
                    if ACT_EXPAND:
                        # exp, expanded across D on the scalar engine so the
                        # DVE multiply gets contiguous fp16 operands
                        el_bc = el[:, c0:c1, :].unsqueeze(3).broadcast_to(
                            [128, w, H, D])
                        nc.scalar.activation(
                            exe[:, c0:c1, :].rearrange(
                                "p i (h d) -> p i h d", d=D),
                            el_bc, mybir.ActivationFunctionType.Exp)
                        nc.vector.tensor_mul(wv[:, c0:c1, 0:FOUT],
                                             vals[:, c0:c1, 0:FOUT],
                                             exe[:, c0:c1, :])
                        nc.vector.tensor_copy(
                            wv[:, c0:c1, FOUT:GCOLS],
                            exe[:, c0:c1, :].rearrange(
                                "p i (h d) -> p i h d", d=D)[:, :, :, 0])
                    else:
                        ex = sm_pool.tile([128, SLOT_I, H], F16, tag="ex")
                        nc.scalar.activation(
                            ex[:, c0:c1, :], el[:, c0:c1, :],
                            mybir.ActivationFunctionType.Exp)
                        ex_bc = ex[:, c0:c1, :].unsqueeze(3).broadcast_to(
                            [128, w, H, D])
                        nc.vector.tensor_mul(
                            wv[:, c0:c1, 0:FOUT].rearrange(
                                "p i (h d) -> p i h d", d=D),
                            vals[:, c0:c1, 0:FOUT].rearrange(
                                "p i (h d) -> p i h d", d=D),
                            ex_bc)
                        nc.vector.tensor_copy(wv[:, c0:c1, FOUT:GCOLS],
                                              ex[:, c0:c1, :])

                active = [i for (c0, c1) in halves for i in range(c0, c1)]
                ps = psum_pool.tile([128, GCOLS], F32)
                for n_i, i in enumerate(active):
                    m = m_pool.tile([128, 128], F16, tag="m")
                    nc.vector.tensor_scalar(
                        m[:], iota_t[:], lct[:, i:i + 1], None,
                        mybir.AluOpType.is_equal)
                    nc.tensor.matmul(ps[:], m[:], wv[:, i, :],
                                     start=(n_i == 0),
                                     stop=(n_i == len(active) - 1))

                # finalize: out = num / max(denom, eps)
                dn = fin_pool.tile([128, H], F32, tag="dn")
                nc.vector.tensor_scalar_max(dn[:], ps[:, FOUT:GCOLS], 1e-12)
                rc = fin_pool.tile([128, H], F32, tag="rc")
                nc.vector.reciprocal(rc[:], dn[:])
                ot = fin_pool.tile([128, FOUT], F16, tag="ot")
                rc_bc = rc[:].unsqueeze(2).broadcast_to([128, H, D])
                nc.vector.tensor_mul(
                    ot[:].rearrange("p (h d) -> p h d", d=D),
                    ps[:, 0:FOUT].rearrange("p (h d) -> p h d", d=D),
                    rc_bc)
                nc.sync.dma_start(outs[b * 128:(b + 1) * 128, :], ot[:])
    nc.compile()
    return nc


# ------------------------------------------------------------ host prep
def plan_blocks(src, dst):
    """Greedy pack dst-sorted edges into blocks with <=128 node span,
    <=CAP_LO lo-edges and <=CAP_HI hi-edges.  Returns block node spans."""
    lo_mask = src < LO_N
    deg_lo = np.bincount(dst[lo_mask], minlength=N)
    deg_hi = np.bincount(dst[~lo_mask], minlength=N)
    blocks = []
    n0 = 0
    clo = chi = 0
    for n in range(N):
        dl, dh = int(deg_lo[n]), int(deg_hi[n])
        if n - n0 >= 128 or clo + dl > CAP_LO or chi + dh > CAP_HI:
            blocks.append((n0, n))
            n0, clo, chi = n, dl, dh
        else:
            clo += dl
            chi += dh
    blocks.append((n0, N))
    return blocks


def wrap16(idx_list, cap):
    full = np.full(cap, -1, dtype=np.int16)
    full[:len(idx_list)] = idx_list
    t = full.reshape(cap // 16, 16).T
    return np.tile(t, (8, 1))


def prep_static(src, dst):
    # stage-1 weights
    def s1_inputs(h, W, bW, a, ba):
        hT = np.zeros((F_IN, NPAD), dtype=np.float16)
        hT[:, :N] = h.T.astype(np.float16)
        waug = np.zeros((F_IN, TCOLS), dtype=np.float16)
        waug[:, :FOUT] = W.transpose(1, 0, 2).reshape(F_IN, FOUT)
        waug[:, FOUT:FOUT + H] = np.einsum("hid,hd->ih", W, a[:, :D])
        waug[:, FOUT + H:] = np.einsum("hid,hd->ih", W, a[:, D:])
        brow = np.zeros(TCOLS, dtype=np.float32)
        brow[:FOUT] = bW.reshape(-1)
        brow[FOUT:FOUT + H] = np.einsum("hd,hd->h", bW, a[:, :D])
        brow[FOUT + H:] = np.einsum("hd,hd->h", bW, a[:, D:]) + ba
        biasb = np.tile(brow, (128, 1)).astype(np.float32)
        return hT, waug, biasb

    blocks = plan_blocks(src, dst)
    nb = len(blocks)
    B = (nb + NCORES - 1) // NCORES
    lo_w = np.full((NCORES, B, 128, CAP_LO // 16), -1, dtype=np.int16)
    hi_w = np.full((NCORES, B, 128, CAP_HI // 16), -1, dtype=np.int16)
    dloc = np.full((NCORES, B, 128, SLOT_I), DUMMY_LOC, dtype=np.float32)
    n_lo = np.zeros((NCORES, B), dtype=np.int64)
    n_hi = np.zeros((NCORES, B), dtype=np.int64)
    # edge slot bookkeeping for the host-side s_dst fill:
    # per (core, block) the global dst index per slot (or -1)
    slot_dst = np.full((NCORES, B, 128, SLOT_I), -1, dtype=np.int64)

    deg = np.bincount(dst, minlength=N)
    starts = np.zeros(N + 1, dtype=np.int64)
    np.cumsum(deg, out=starts[1:])
    lo_mask = src < LO_N

    per_block = {}
    for j, (ns, ne) in enumerate(blocks):
        c, bb = j % NCORES, j // NCORES
        es, ee = int(starts[ns]), int(starts[ne])
        bsrc = src[es:ee]
        bdst = dst[es:ee]
        bm = lo_mask[es:ee]
        per_block[(c, bb)] = (ns, bsrc, bdst, bm)
        n_lo[c, bb] = int(bm.sum())
        n_hi[c, bb] = int((~bm).sum())
    # SPMD shares one program, so num_idxs_reg must be identical across
    # cores: use the per-block max rounded up to whole chunks (chunk
    # activity is compile-time).  Shorter cores pad with TRAILING -1
    # indices, which the gather ucode trims at runtime, so each core only
    # moves ceil(own_count/128)*128 rows.  Untrimmed garbage slots keep
    # stale SBUF (zero-initialized at start); their sdst is -300 so
    # exp() underflows to 0 and dloc stays DUMMY_LOC (one-hot miss).
    n_lo_u = (n_lo.max(axis=0) + 127) // 128 * 128
    n_hi_u = (n_hi.max(axis=0) + 127) // 128 * 128
    empty = (0, np.zeros(0, dtype=np.int64), np.zeros(0, dtype=np.int64),
             np.zeros(0, dtype=bool))
    for c in range(NCORES):
      for bb in range(B):
        ns, bsrc, bdst, bm = per_block.get((c, bb), empty)
        for half, (sel, base, cap, coff, n_u) in enumerate(
                [(bm, 0, CAP_LO, 0, n_lo_u[bb]),
                 (~bm, LO_N, CAP_HI, C_LO, n_hi_u[bb])]):
            hsrc = (bsrc[sel] - base).astype(np.int16)
            hdst = bdst[sel]
            cnt = len(hsrc)
            padded = np.zeros(int(n_u), dtype=np.int16)
            padded[:cnt] = hsrc
            if half == 0:
                lo_w[c, bb] = wrap16(padded, CAP_LO)
            else:
                hi_w[c, bb] = wrap16(padded, CAP_HI)
            s = np.arange(cnt)
            k, i = s % 128, coff + s // 128
            dloc[c, bb, k, i] = (hdst - ns).astype(np.float32)
            slot_dst[c, bb, k, i] = hdst
    iota = np.tile(np.arange(128, dtype=np.float16), (128, 1))
    return (s1_inputs, blocks, B, lo_w, hi_w, dloc, n_lo_u, n_hi_u,
            slot_dst, iota)


def make_sdst(tfull, slot_dst):
    """Host-side fill of per-edge-slot s_dst (+ba) from the stage-1 table.

    Invalid slots get -300 so exp(leakyrelu(s_src + s_dst)) underflows to
    exactly 0 in fp16, zeroing any stale gathered values they multiply."""
    sd_tab = tfull[:, GCOLS:TCOLS]                      # [NPAD, 4] fp16
    flat = slot_dst.reshape(-1)
    out = np.full((flat.shape[0], H), -300.0, dtype=np.float16)
    valid = flat >= 0
    out[valid] = sd_tab[flat[valid]]
    return out.reshape(slot_dst.shape + (H,))


# ------------------------------------------------------------------ main
def kernel(h, W, bW, a, ba, src, dst):
    h = np.asarray(h, dtype=np.float32)
    W = np.asarray(W, dtype=np.float32)
    bW = np.asarray(bW, dtype=np.float32)
    a = np.asarray(a, dtype=np.float32)
    ba = np.asarray(ba, dtype=np.float32)
    src = np.asarray(src, dtype=np.int64)
    dst = np.asarray(dst, dtype=np.int64)

    (s1_inputs, blocks, B, lo_w, hi_w, dloc, n_lo_u, n_hi_u, slot_dst,
     iota) = prep_static(src, dst)
    hT, waug, biasb = s1_inputs(h, W, bW, a, ba)
    core_ids = list(range(NCORES))

    if "s1" not in _cache:
        _cache["s1"] = build_stage1()
    nc1 = _cache["s1"]
    ins1 = [{"hT": np.ascontiguousarray(hT[:, c * NSLICE:(c + 1) * NSLICE]),
             "waug": waug, "biasb": biasb} for c in core_ids]
    res1 = run_bass_kernel_spmd(nc1, ins1, core_ids)
    tfull = np.concatenate([res1.results[c]["tout"] for c in core_ids], axis=0)

    sdst = make_sdst(tfull, slot_dst)

    key = ("s2", B, hash((tuple(n_lo_u), tuple(n_hi_u))))
    if key not in _cache:
        _cache[key] = build_stage2(B, n_lo_u, n_hi_u)
    nc2 = _cache[key]
    ins2 = [{"tfull": tfull, "lowi": lo_w[c], "hiwi": hi_w[c],
             "dloc": dloc[c], "sdst": sdst[c], "iota": iota}
            for c in core_ids]
    res2 = run_bass_kernel_spmd(nc2, ins2, core_ids)

    out = np.zeros((N, FOUT), dtype=np.float32)
    for j, (ns, ne) in enumerate(blocks):
        c, bb = j % NCORES, j // NCORES
        out[ns:ne] = res2.results[c]["outs"][
            bb * 128:bb * 128 + (ne - ns), :].astype(np.float32)
    return out


# ------------------------------------------------------------- benchmarking
def _make_runner(nc, in_maps):
    """Repeatedly-callable jitted shard_map for a compiled Bass program with
    device-resident inputs (mirrors bass2jax.run_bass_via_pjrt, minus output
    donation so buffers can be reused across timing iterations)."""
    import jax
    from jax.sharding import Mesh, PartitionSpec, NamedSharding
    from jax.experimental.shard_map import shard_map
    from concourse import bass2jax
    from concourse.bass2jax import _bass_exec_p, partition_id_tensor

    bass2jax.install_neuronx_cc_hook()
    n_cores = len(in_maps)
    part_name = (nc.partition_id_tensor.name
                 if nc.partition_id_tensor else None)
    in_names, out_names, out_avals, zero_outs = [], [], [], []
    for alloc in nc.m.functions[0].allocations:
        if not isinstance(alloc, mybir.MemoryLocationSet):
            continue
        name = alloc.memorylocations[0].name
        if alloc.kind == "ExternalInput":
            if name != part_name:
                in_names.append(name)
        elif alloc.kind == "ExternalOutput":
            out_names.append(name)
            shape = tuple(alloc.tensor_shape)
            dtype = mybir.dt.np(alloc.dtype)
            out_avals.append(jax.core.ShapedArray(shape, dtype))
            zero_outs.append(np.zeros(shape, dtype))
    n_params = len(in_names)
    all_names = list(in_names) + out_names
    if part_name is not None:
        all_names.append(part_name)

    def _body(*args):
        operands = list(args)
        if part_name is not None:
            operands.append(partition_id_tensor())
        outs = _bass_exec_p.bind(
            *operands,
            out_avals=tuple(out_avals),
            in_names=tuple(all_names),
            out_names=tuple(out_names),
            lowering_input_output_aliases=(),
            sim_require_finite=True,
            sim_require_nnan=True,
            nc=nc,
        )
        return tuple(outs)

    devices = jax.devices()[:n_cores]
    mesh = Mesh(np.asarray(devices), ("core",))
    spec = NamedSharding(mesh, PartitionSpec("core"))
    in_specs = (PartitionSpec("core"),) * (n_params + len(out_names))
    out_specs = (PartitionSpec("core"),) * len(out_names)
    fn = jax.jit(shard_map(_body, mesh=mesh, in_specs=in_specs,
                           out_specs=out_specs, check_rep=False),
                 keep_unused=True)
    args = []
    for name in in_names:
        cat = np.concatenate([np.asarray(m[name]) for m in in_maps], axis=0)
        args.append(jax.device_put(cat, spec))
    for z in zero_outs:
        cat = np.concatenate([z] * n_cores, axis=0)
        args.append(jax.device_put(cat, spec))
    return fn, args, out_names


def build_noop():
    nc = bacc.Bacc("TRN2", target_bir_lowering=False, debug=False,
                   num_devices=NCORES)
    x = nc.dram_tensor("x", [128, 128], F32, kind="ExternalInput").ap()
    y = nc.dram_tensor("y", [128, 128], F32, kind="ExternalOutput").ap()
    with tile.TileContext(nc) as tc:
        with ExitStack() as ctx:
            pool = ctx.enter_context(tc.tile_pool(name="p", bufs=1))
            t = pool.tile([128, 128], F32)
            nc.sync.dma_start(t[:], x[:])
            nc.sync.dma_start(y[:], t[:])
    nc.compile()
    return nc


def bench(inputs, iters=5):
    """Steady-state wall-clock ns for one launch of stage1+stage2 with all
    inputs resident on the NeuronCores (no NTFF profiling under axon)."""
    import jax, time

    h = np.asarray(inputs["h"], dtype=np.float32)
    W = np.asarray(inputs["W"], dtype=np.float32)
    bW = np.asarray(inputs["bW"], dtype=np.float32)
    a = np.asarray(inputs["a"], dtype=np.float32)
    ba = np.asarray(inputs["ba"], dtype=np.float32)
    src = np.asarray(inputs["src"], dtype=np.int64)
    dst = np.asarray(inputs["dst"], dtype=np.int64)
    (s1_inputs, blocks, B, lo_w, hi_w, dloc, n_lo_u, n_hi_u, slot_dst,
     iota) = prep_static(src, dst)
    hT, waug, biasb = s1_inputs(h, W, bW, a, ba)
    core_ids = list(range(NCORES))

    if "s1" not in _cache:
        _cache["s1"] = build_stage1()
    nc1 = _cache["s1"]
    ins1 = [{"hT": np.ascontiguousarray(hT[:, c * NSLICE:(c + 1) * NSLICE]),
             "waug": waug, "biasb": biasb} for c in core_ids]
    fn1, args1, onames1 = _make_runner(nc1, ins1)
    o1 = fn1(*args1)
    jax.block_until_ready(o1)
    tcat = np.asarray(o1[onames1.index("tout")])
    tfull = tcat.reshape(NCORES * NSLICE, TROW)
    sdst = make_sdst(tfull, slot_dst)

    key = ("s2", B, hash((tuple(n_lo_u), tuple(n_hi_u))))
    if key not in _cache:
        _cache[key] = build_stage2(B, n_lo_u, n_hi_u)
    nc2 = _cache[key]
    ins2 = [{"tfull": tfull, "lowi": lo_w[c], "hiwi": hi_w[c],
             "dloc": dloc[c], "sdst": sdst[c], "iota": iota}
            for c in core_ids]
    fn2, args2, _ = _make_runner(nc2, ins2)
    jax.block_until_ready(fn2(*args2))

    if "noop" not in _cache:
        _cache["noop"] = build_noop()
    ncn = _cache["noop"]
    insn = [{"x": np.zeros((128, 128), np.float32)} for _ in core_ids]
    fnn, argsn, _ = _make_runner(ncn, insn)
    jax.block_until_ready(fnn(*argsn))

    def once(fn, args):
        t0 = time.perf_counter()
        jax.block_until_ready(fn(*args))
        return time.perf_counter() - t0

    R = 4
    if ("s1r", R) not in _cache:
        _cache[("s1r", R)] = build_stage1(repeat=R)
    fn1r, args1r, _ = _make_runner(_cache[("s1r", R)], ins1)
    jax.block_until_ready(fn1r(*args1r))
    if (key, R) not in _cache:
        _cache[(key, R)] = build_stage2(B, n_lo_u, n_hi_u, repeat=R)
    fn2r, args2r, _ = _make_runner(_cache[(key, R)], ins2)
    jax.block_until_ready(fn2r(*args2r))

    t_noop = [once(fnn, argsn) for _ in range(iters)]
    t_s1 = [once(fn1, args1) for _ in range(iters)]
    t_s2 = [once(fn2, args2) for _ in range(iters)]
    t_s1r = [once(fn1r, args1r) for _ in range(iters)]
    t_s2r = [once(fn2r, args2r) for _ in range(iters)]
    f = min(t_noop)
    s1, s2 = min(t_s1), min(t_s2)
    s1r, s2r = min(t_s1r), min(t_s2r)
    e1 = (s1r - s1) / (R - 1)
    e2 = (s2r - s2) / (R - 1)
    print(f"  walls us: noop {f*1e6:.0f}  s1 {s1*1e6:.0f}  s1x{R} {s1r*1e6:.0f}"
          f"  s2 {s2*1e6:.0f}  s2x{R} {s2r*1e6:.0f}")
    print(f"  per-iter device exec: stage1 {e1*1e9:.0f} ns  "
          f"stage2 {e2*1e9:.0f} ns")
    est = max(e1, 0.0) + max(e2, 0.0)
    print(f"  estimated device exec total: {est*1e9:.0f} ns")
    return int(est * 1e9)



# revision 18
# speedup vs baseline: 1.9041x; 1.9041x over previous
"""GAT layer kernel for Trainium2, sharded across 8 NeuronCores.

Strategy (edge partitioning, per sharding hint):
  Stage 1 (node-sharded SPMD): per-head projection Wh = h @ W + bW fused with
    the attention partials s_src = Wh.a_src, s_dst = Wh.a_dst + ba via one
    augmented matmul per 128-node chunk -> fp16 table
    T[n] = [Wh (128) | s_src (4) | s_dst (4) | pad] (512B rows).
  Host: concat T slices, build the per-core edge-block schedule.
  Stage 2 (edge-sharded SPMD): edges grouped by destination into blocks
    (<=128 dst nodes, 16 chunks x 128 edge slots; chunks 0-7 hold edges with
    src < 25088, chunks 8-15 the rest, so gather indices fit in int16).
    Per block: two dma_gather ops pull T[src] rows, alpha =
    exp(leakyrelu(s_src + s_dst)) is computed on DVE/ACT, gathered Wh rows are
    scaled by alpha, and per-chunk one-hot matmuls scatter-add [num | denom]
    into a [dst_node, 132] PSUM accumulator.  Softmax max-subtraction is
    skipped (|e| = O(1) and softmax is shift-invariant).
    Finalize: out = num * (1/denom).
"""

import numpy as np
from contextlib import ExitStack

from concourse import bass, bacc, tile
import concourse.mybir as mybir
from concourse.bass_utils import run_bass_kernel_spmd

F32 = mybir.dt.float32
F16 = mybir.dt.float16
I16 = mybir.dt.int16

# Problem constants (nn_GAT_36009005809883)
N, E, F_IN, H, D = 50000, 800000, 256, 4, 32
ALPHA = 0.2
NCORES = 8
FOUT = H * D             # 128
TCOLS = FOUT + 2 * H     # 136 = Wh | s_src | s_dst
GCOLS = FOUT + H         # 132 used per edge: Wh | s_src
TROW = 256               # fp16 table row (512B)
NPAD = 50176             # 392 chunks of 128 nodes, 49 per core
NSLICE = NPAD // NCORES  # 6272
LO_N = 25088             # table split: both halves' indices fit int16
C_LO, C_HI = 8, 8        # dma_gather num_idxs is capped at 1024
SLOT_I = C_LO + C_HI     # 17 chunks per block
CAP_LO, CAP_HI = C_LO * 128, C_HI * 128
DUMMY_LOC = 300.0

_cache = {}
ACT_EXPAND = True


# ----------------------------------------------------------------- stage 1
def build_stage1(repeat=1):
    """Projection. All-SBUF-resident: one DMA in (hT), one DMA out
    (partition-major table: node n = c*128+p lives at tout[p, c, :]).
    Per-chunk matmuls read/write SBUF only, so there are no small
    per-chunk HBM descriptors (those dominated the old 83us)."""
    nc = bacc.Bacc("TRN2", target_bir_lowering=False, debug=False,
                   num_devices=NCORES)
    hT = nc.dram_tensor("hT", [F_IN, NSLICE], F16, kind="ExternalInput").ap()
    waug = nc.dram_tensor("waug", [F_IN, TCOLS], F16, kind="ExternalInput").ap()
    biasb = nc.dram_tensor("biasb", [128, TCOLS], F32, kind="ExternalInput").ap()
    nchunks = NSLICE // 128
    tout = nc.dram_tensor("tout", [128, nchunks, TROW], F16,
                          kind="ExternalOutput").ap()

    with tile.TileContext(nc) as tc:
        with ExitStack() as ctx:
            consts = ctx.enter_context(tc.tile_pool(name="consts", bufs=1))
            psum_pool = ctx.enter_context(
                tc.tile_pool(name="ps", bufs=4, space="PSUM"))

            w_t = consts.tile([128, 2, TCOLS], F16)
            nc.sync.dma_start(w_t[:], waug.rearrange("(k p) f -> p k f", p=128))
            b_t = consts.tile([128, TCOLS], F32)
            nc.sync.dma_start(b_t[:], biasb[:])
            h_sb = consts.tile([128, 2, NSLICE], F16)
            nc.sync.dma_start(h_sb[:], hT.rearrange("(k p) n -> p k n", p=128))
            o_sb = consts.tile([128, nchunks, TROW], F16)

            for _r in range(repeat):
              for c in range(nchunks):
                ps = psum_pool.tile([128, TCOLS], F32)
                for kt in range(2):
                    nc.tensor.matmul(ps[:], h_sb[:, kt, c * 128:(c + 1) * 128],
                                     w_t[:, kt, :],
                                     start=(kt == 0), stop=(kt == 1))
                nc.vector.tensor_add(o_sb[:, c, 0:TCOLS], ps[:], b_t[:])
              nc.sync.dma_start(tout[:], o_sb[:])
    nc.compile()
    return nc


# ----------------------------------------------------------------- stage 2
def build_stage2(B, n_lo, n_hi, repeat=1, act_expand=None):
    global ACT_EXPAND
    if act_expand is not None:
        ACT_EXPAND = act_expand
    nc = bacc.Bacc("TRN2", target_bir_lowering=False, debug=False,
                   num_devices=NCORES)
    tfull = nc.dram_tensor("tfull", [NPAD, TROW], F16, kind="ExternalInput").ap()
    lowi = nc.dram_tensor("lowi", [128, B * (CAP_LO // 16)], I16,
                          kind="ExternalInput").ap()
    hiwi = nc.dram_tensor("hiwi", [128, B * (CAP_HI // 16)], I16,
                          kind="ExternalInput").ap()
    dloc = nc.dram_tensor("dloc", [128, B * SLOT_I], F32,
                          kind="ExternalInput").ap()
    sdst = nc.dram_tensor("sdst", [128, B * SLOT_I * H], F16,
                          kind="ExternalInput").ap()
    iota = nc.dram_tensor("iota", [128, 128], F16, kind="ExternalInput").ap()
    outs = nc.dram_tensor("outs", [128, B * FOUT], F16,
                          kind="ExternalOutput").ap()

    tlo = tfull[0:LO_N, :]
    thi = tfull[LO_N:NPAD, :]

    with tile.TileContext(nc) as tc:
        with ExitStack() as ctx:
            consts = ctx.enter_context(tc.tile_pool(name="consts", bufs=1))
            val_pool = ctx.enter_context(tc.tile_pool(name="vals", bufs=3))
            wv_pool = ctx.enter_context(tc.tile_pool(name="wv", bufs=3))
            sm_pool = ctx.enter_context(tc.tile_pool(name="small", bufs=3))
            m_pool = ctx.enter_context(tc.tile_pool(name="oneh", bufs=2 * SLOT_I))
            fin_pool = ctx.enter_context(tc.tile_pool(name="fin", bufs=3))
            psum_pool = ctx.enter_context(
                tc.tile_pool(name="ps", bufs=2, space="PSUM"))

            iota_t = consts.tile([128, 128], F16)
            nc.sync.dma_start(iota_t[:], iota[:])
            # whole-kernel resident inputs/outputs: one large contiguous DMA
            # each instead of per-block small-descriptor transfers.
            lo_sb = consts.tile([128, B, CAP_LO // 16], I16)
            nc.sync.dma_start(lo_sb[:], lowi.rearrange("p (b w) -> p b w", b=B))
            hi_sb = consts.tile([128, B, CAP_HI // 16], I16)
            nc.sync.dma_start(hi_sb[:], hiwi.rearrange("p (b w) -> p b w", b=B))
            lc_sb = consts.tile([128, B, SLOT_I], F32)
            nc.sync.dma_start(lc_sb[:], dloc.rearrange("p (b i) -> p b i", b=B))
            sd_sb = consts.tile([128, B, SLOT_I, H], F16)
            nc.sync.dma_start(sd_sb[:],
                              sdst.rearrange("p (b i h) -> p b i h", b=B, h=H))
            o_sb = consts.tile([128, B, FOUT], F16)
            nc.vector.memset(o_sb[:], 0.0)

            # zero-init every vals buffer once: gathers of padded blocks can
            # leave slots unwritten, and SBUF garbage at kernel start could
            # be NaN (NaN * 0 would poison the PSUM accum).
            for _i in range(3):
                vz = val_pool.tile([128, SLOT_I, TROW], F16)
                nc.vector.memset(vz[:], 0.0)

            for _r in range(repeat):
              for b in range(B):
                # active chunks (counts are pre-rounded to multiples of 128,
                # so every active chunk is fully gathered -- compute never
                # touches uninitialized SBUF, which ACT would trap on)
                a_lo = int(n_lo[b]) // 128
                a_hi = int(n_hi[b]) // 128
                halves = []
                if a_lo:
                    halves.append((0, a_lo))
                if a_hi:
                    halves.append((C_LO, C_LO + a_hi))
                if not halves:
                    continue

                lct = lc_sb[:, b, :]
                sdt = sd_sb[:, b, :, :]

                vals = val_pool.tile([128, SLOT_I, TROW], F16)
                if a_lo:
                    nc.gpsimd.dma_gather(vals[:, 0:C_LO, :], tlo,
                                         lo_sb[:, b, :],
                                         CAP_LO, int(n_lo[b]), TROW)
                if a_hi:
                    nc.gpsimd.dma_gather(vals[:, C_LO:SLOT_I, :], thi,
                                         hi_sb[:, b, :],
                                         CAP_HI, int(n_hi[b]), TROW)

                # e = s_src + s_dst ; leaky ; exp ; weighted values
                # (restricted to active chunks per half)
                ev = sm_pool.tile([128, SLOT_I, H], F32, tag="ev")
                t1 = sm_pool.tile([128, SLOT_I, H], F32, tag="t1")
                el = sm_pool.tile([128, SLOT_I, H], F32, tag="el")
                exe = sm_pool.tile([128, SLOT_I, FOUT], F16, tag="exe")
                wv = wv_pool.tile([128, SLOT_I, GCOLS], F16)
                for (c0, c1) in halves:
                    w = c1 - c0
                    nc.vector.tensor_add(ev[:, c0:c1, :],
                                         vals[:, c0:c1, FOUT:GCOLS],
                                         sdt[:, c0:c1, :])
                    nc.vector.tensor_scalar_mul(t1[:, c0:c1, :],
                                                ev[:, c0:c1, :], ALPHA)
                    nc.vector.tensor_max(el[:, c0:c1, :], ev[:, c0:c1, :],
                                         t1[:, c0:c1, :])
                    if ACT_EXPAND:
                        # exp, expanded across D on the scalar engine so the
                        # DVE multiply gets contiguous fp16 operands
                        el_bc = el[:, c0:c1, :].unsqueeze(3).broadcast_to(
                            [128, w, H, D])
                        nc.scalar.activation(
                            exe[:, c0:c1, :].rearrange(
                                "p i (h d) -> p i h d", d=D),
                            el_bc, mybir.ActivationFunctionType.Exp)
                        nc.vector.tensor_mul(wv[:, c0:c1, 0:FOUT],
                                             vals[:, c0:c1, 0:FOUT],
                                             exe[:, c0:c1, :])
                        nc.vector.tensor_copy(
                            wv[:, c0:c1, FOUT:GCOLS],
                            exe[:, c0:c1, :].rearrange(
                                "p i (h d) -> p i h d", d=D)[:, :, :, 0])
                    else:
                        ex = sm_pool.tile([128, SLOT_I, H], F16, tag="ex")
                        nc.scalar.activation(
                            ex[:, c0:c1, :], el[:, c0:c1, :],
                            mybir.ActivationFunctionType.Exp)
                        ex_bc = ex[:, c0:c1, :].unsqueeze(3).broadcast_to(
                            [128, w, H, D])
                        nc.vector.tensor_mul(
                            wv[:, c0:c1, 0:FOUT].rearrange(
                                "p i (h d) -> p i h d", d=D),
                            vals[:, c0:c1, 0:FOUT].rearrange(
                                "p i (h d) -> p i h d", d=D),
                            ex_bc)
                        nc.vector.tensor_copy(wv[:, c0:c1, FOUT:GCOLS],
                                              ex[:, c0:c1, :])

                active = [i for (c0, c1) in halves for i in range(c0, c1)]
                ps = psum_pool.tile([128, GCOLS], F32)
                for n_i, i in enumerate(active):
                    m = m_pool.tile([128, 128], F16, tag="m")
                    nc.vector.tensor_scalar(
                        m[:], iota_t[:], lct[:, i:i + 1], None,
                        mybir.AluOpType.is_equal)
                    nc.tensor.matmul(ps[:], m[:], wv[:, i, :],
                                     start=(n_i == 0),
                                     stop=(n_i == len(active) - 1))

                # finalize: out = num / max(denom, eps)
                dn = fin_pool.tile([128, H], F32, tag="dn")
                nc.vector.tensor_scalar_max(dn[:], ps[:, FOUT:GCOLS], 1e-12)
                rc = fin_pool.tile([128, H], F32, tag="rc")
                nc.vector.reciprocal(rc[:], dn[:])
                rc_bc = rc[:].unsqueeze(2).broadcast_to([128, H, D])
                nc.vector.tensor_mul(
                    o_sb[:, b, :].rearrange("p (h d) -> p h d", d=D),
                    ps[:, 0:FOUT].rearrange("p (h d) -> p h d", d=D),
                    rc_bc)
              nc.sync.dma_start(outs[:], o_sb[:])
    nc.compile()
    return nc


# ------------------------------------------------------------ host prep
def plan_blocks(src, dst):
    """Greedy pack dst-sorted edges into blocks with <=128 node span,
    <=CAP_LO lo-edges and <=CAP_HI hi-edges.  Returns block node spans."""
    lo_mask = src < LO_N
    deg_lo = np.bincount(dst[lo_mask], minlength=N)
    deg_hi = np.bincount(dst[~lo_mask], minlength=N)
    blocks = []
    n0 = 0
    clo = chi = 0
    for n in range(N):
        dl, dh = int(deg_lo[n]), int(deg_hi[n])
        if n - n0 >= 128 or clo + dl > CAP_LO or chi + dh > CAP_HI:
            blocks.append((n0, n))
            n0, clo, chi = n, dl, dh
        else:
            clo += dl
            chi += dh
    blocks.append((n0, N))
    return blocks


def wrap16(idx_list, cap):
    full = np.full(cap, -1, dtype=np.int16)
    full[:len(idx_list)] = idx_list
    t = full.reshape(cap // 16, 16).T
    return np.tile(t, (8, 1))


def prep_static(src, dst):
    # stage-1 weights
    def s1_inputs(h, W, bW, a, ba):
        hT = np.zeros((F_IN, NPAD), dtype=np.float16)
        hT[:, :N] = h.T.astype(np.float16)
        waug = np.zeros((F_IN, TCOLS), dtype=np.float16)
        waug[:, :FOUT] = W.transpose(1, 0, 2).reshape(F_IN, FOUT)
        waug[:, FOUT:FOUT + H] = np.einsum("hid,hd->ih", W, a[:, :D])
        waug[:, FOUT + H:] = np.einsum("hid,hd->ih", W, a[:, D:])
        brow = np.zeros(TCOLS, dtype=np.float32)
        brow[:FOUT] = bW.reshape(-1)
        brow[FOUT:FOUT + H] = np.einsum("hd,hd->h", bW, a[:, :D])
        brow[FOUT + H:] = np.einsum("hd,hd->h", bW, a[:, D:]) + ba
        biasb = np.tile(brow, (128, 1)).astype(np.float32)
        return hT, waug, biasb

    blocks = plan_blocks(src, dst)
    nb = len(blocks)
    B = (nb + NCORES - 1) // NCORES
    lo_w = np.full((NCORES, B, 128, CAP_LO // 16), -1, dtype=np.int16)
    hi_w = np.full((NCORES, B, 128, CAP_HI // 16), -1, dtype=np.int16)
    dloc = np.full((NCORES, B, 128, SLOT_I), DUMMY_LOC, dtype=np.float32)
    n_lo = np.zeros((NCORES, B), dtype=np.int64)
    n_hi = np.zeros((NCORES, B), dtype=np.int64)
    # edge slot bookkeeping for the host-side s_dst fill:
    # per (core, block) the global dst index per slot (or -1)
    slot_dst = np.full((NCORES, B, 128, SLOT_I), -1, dtype=np.int64)

    deg = np.bincount(dst, minlength=N)
    starts = np.zeros(N + 1, dtype=np.int64)
    np.cumsum(deg, out=starts[1:])
    lo_mask = src < LO_N

    per_block = {}
    for j, (ns, ne) in enumerate(blocks):
        c, bb = j % NCORES, j // NCORES
        es, ee = int(starts[ns]), int(starts[ne])
        bsrc = src[es:ee]
        bdst = dst[es:ee]
        bm = lo_mask[es:ee]
        per_block[(c, bb)] = (ns, bsrc, bdst, bm)
        n_lo[c, bb] = int(bm.sum())
        n_hi[c, bb] = int((~bm).sum())
    # SPMD shares one program, so num_idxs_reg must be identical across
    # cores: use the per-block max rounded up to whole chunks (chunk
    # activity is compile-time).  Shorter cores pad with TRAILING -1
    # indices, which the gather ucode trims at runtime, so each core only
    # moves ceil(own_count/128)*128 rows.  Untrimmed garbage slots keep
    # stale SBUF (zero-initialized at start); their sdst is -300 so
    # exp() underflows to 0 and dloc stays DUMMY_LOC (one-hot miss).
    n_lo_u = (n_lo.max(axis=0) + 127) // 128 * 128
    n_hi_u = (n_hi.max(axis=0) + 127) // 128 * 128
    empty = (0, np.zeros(0, dtype=np.int64), np.zeros(0, dtype=np.int64),
             np.zeros(0, dtype=bool))
    for c in range(NCORES):
      for bb in range(B):
        ns, bsrc, bdst, bm = per_block.get((c, bb), empty)
        for half, (sel, base, cap, coff, n_u) in enumerate(
                [(bm, 0, CAP_LO, 0, n_lo_u[bb]),
                 (~bm, LO_N, CAP_HI, C_LO, n_hi_u[bb])]):
            hsrc = (bsrc[sel] - base).astype(np.int16)
            hdst = bdst[sel]
            cnt = len(hsrc)
            padded = np.zeros(int(n_u), dtype=np.int16)
            padded[:cnt] = hsrc
            if half == 0:
                lo_w[c, bb] = wrap16(padded, CAP_LO)
            else:
                hi_w[c, bb] = wrap16(padded, CAP_HI)
            s = np.arange(cnt)
            k, i = s % 128, coff + s // 128
            dloc[c, bb, k, i] = (hdst - ns).astype(np.float32)
            slot_dst[c, bb, k, i] = hdst
    iota = np.tile(np.arange(128, dtype=np.float16), (128, 1))
    return (s1_inputs, blocks, B, lo_w, hi_w, dloc, n_lo_u, n_hi_u,
            slot_dst, iota)


def make_sdst(tfull, slot_dst):
    """Host-side fill of per-edge-slot s_dst (+ba) from the stage-1 table.

    Invalid slots get -300 so exp(leakyrelu(s_src + s_dst)) underflows to
    exactly 0 in fp16, zeroing any stale gathered values they multiply."""
    sd_tab = tfull[:, GCOLS:TCOLS]                      # [NPAD, 4] fp16
    flat = slot_dst.reshape(-1)
    out = np.full((flat.shape[0], H), -300.0, dtype=np.float16)
    valid = flat >= 0
    out[valid] = sd_tab[flat[valid]]
    return out.reshape(slot_dst.shape + (H,))


def core_inputs2(tfull, lo_w, hi_w, dloc, sdst, iota, c):
    """Flatten per-core stage-2 inputs to the [128, B*...] resident layouts."""
    B = lo_w.shape[1]
    return {
        "tfull": tfull,
        "lowi": np.ascontiguousarray(
            lo_w[c].transpose(1, 0, 2).reshape(128, -1)),
        "hiwi": np.ascontiguousarray(
            hi_w[c].transpose(1, 0, 2).reshape(128, -1)),
        "dloc": np.ascontiguousarray(
            dloc[c].transpose(1, 0, 2).reshape(128, -1)),
        "sdst": np.ascontiguousarray(
            sdst[c].transpose(1, 0, 2, 3).reshape(128, -1)),
        "iota": iota,
    }


def tout_to_rows(tout_pm):
    """[128, nchunks, TROW] partition-major -> [NSLICE, TROW] node rows."""
    return np.ascontiguousarray(
        tout_pm.transpose(1, 0, 2).reshape(-1, TROW))


def outs_to_blocks(outs_pm, B):
    """[128, B*FOUT] -> [B, 128, FOUT]."""
    return outs_pm.reshape(128, B, FOUT).transpose(1, 0, 2)


# ------------------------------------------------------------------ main
def kernel(h, W, bW, a, ba, src, dst):
    h = np.asarray(h, dtype=np.float32)
    W = np.asarray(W, dtype=np.float32)
    bW = np.asarray(bW, dtype=np.float32)
    a = np.asarray(a, dtype=np.float32)
    ba = np.asarray(ba, dtype=np.float32)
    src = np.asarray(src, dtype=np.int64)
    dst = np.asarray(dst, dtype=np.int64)

    (s1_inputs, blocks, B, lo_w, hi_w, dloc, n_lo_u, n_hi_u, slot_dst,
     iota) = prep_static(src, dst)
    hT, waug, biasb = s1_inputs(h, W, bW, a, ba)
    core_ids = list(range(NCORES))

    if "s1" not in _cache:
        _cache["s1"] = build_stage1()
    nc1 = _cache["s1"]
    ins1 = [{"hT": np.ascontiguousarray(hT[:, c * NSLICE:(c + 1) * NSLICE]),
             "waug": waug, "biasb": biasb} for c in core_ids]
    res1 = run_bass_kernel_spmd(nc1, ins1, core_ids)
    tfull = np.concatenate(
        [tout_to_rows(res1.results[c]["tout"]) for c in core_ids], axis=0)

    sdst = make_sdst(tfull, slot_dst)

    key = ("s2", B, hash((tuple(n_lo_u), tuple(n_hi_u))))
    if key not in _cache:
        _cache[key] = build_stage2(B, n_lo_u, n_hi_u)
    nc2 = _cache[key]
    ins2 = [core_inputs2(tfull, lo_w, hi_w, dloc, sdst, iota, c)
            for c in core_ids]
    res2 = run_bass_kernel_spmd(nc2, ins2, core_ids)

    out = np.zeros((N, FOUT), dtype=np.float32)
    obl = {c: outs_to_blocks(res2.results[c]["outs"], B) for c in core_ids}
    for j, (ns, ne) in enumerate(blocks):
        c, bb = j % NCORES, j // NCORES
        out[ns:ne] = obl[c][bb, 0:ne - ns, :].astype(np.float32)
    return out


# ------------------------------------------------------------- benchmarking
def _make_runner(nc, in_maps):
    """Repeatedly-callable jitted shard_map for a compiled Bass program with
    device-resident inputs (mirrors bass2jax.run_bass_via_pjrt, minus output
    donation so buffers can be reused across timing iterations)."""
    import jax
    from jax.sharding import Mesh, PartitionSpec, NamedSharding
    from jax.experimental.shard_map import shard_map
    from concourse import bass2jax
    from concourse.bass2jax import _bass_exec_p, partition_id_tensor

    bass2jax.install_neuronx_cc_hook()
    n_cores = len(in_maps)
    part_name = (nc.partition_id_tensor.name
                 if nc.partition_id_tensor else None)
    in_names, out_names, out_avals, zero_outs = [], [], [], []
    for alloc in nc.m.functions[0].allocations:
        if not isinstance(alloc, mybir.MemoryLocationSet):
            continue
        name = alloc.memorylocations[0].name
        if alloc.kind == "ExternalInput":
            if name != part_name:
                in_names.append(name)
        elif alloc.kind == "ExternalOutput":
            out_names.append(name)
            shape = tuple(alloc.tensor_shape)
            dtype = mybir.dt.np(alloc.dtype)
            out_avals.append(jax.core.ShapedArray(shape, dtype))
            zero_outs.append(np.zeros(shape, dtype))
    n_params = len(in_names)
    all_names = list(in_names) + out_names
    if part_name is not None:
        all_names.append(part_name)

    def _body(*args):
        operands = list(args)
        if part_name is not None:
            operands.append(partition_id_tensor())
        outs = _bass_exec_p.bind(
            *operands,
            out_avals=tuple(out_avals),
            in_names=tuple(all_names),
            out_names=tuple(out_names),
            lowering_input_output_aliases=(),
            sim_require_finite=True,
            sim_require_nnan=True,
            nc=nc,
        )
        return tuple(outs)

    devices = jax.devices()[:n_cores]
    mesh = Mesh(np.asarray(devices), ("core",))
    spec = NamedSharding(mesh, PartitionSpec("core"))
    in_specs = (PartitionSpec("core"),) * (n_params + len(out_names))
    out_specs = (PartitionSpec("core"),) * len(out_names)
    fn = jax.jit(shard_map(_body, mesh=mesh, in_specs=in_specs,
                           out_specs=out_specs, check_rep=False),
                 keep_unused=True)
    args = []
    for name in in_names:
        cat = np.concatenate([np.asarray(m[name]) for m in in_maps], axis=0)
        args.append(jax.device_put(cat, spec))
    for z in zero_outs:
        cat = np.concatenate([z] * n_cores, axis=0)
        args.append(jax.device_put(cat, spec))
    return fn, args, out_names


def build_noop():
    nc = bacc.Bacc("TRN2", target_bir_lowering=False, debug=False,
                   num_devices=NCORES)
    x = nc.dram_tensor("x", [128, 128], F32, kind="ExternalInput").ap()
    y = nc.dram_tensor("y", [128, 128], F32, kind="ExternalOutput").ap()
    with tile.TileContext(nc) as tc:
        with ExitStack() as ctx:
            pool = ctx.enter_context(tc.tile_pool(name="p", bufs=1))
            t = pool.tile([128, 128], F32)
            nc.sync.dma_start(t[:], x[:])
            nc.sync.dma_start(y[:], t[:])
    nc.compile()
    return nc


def bench(inputs, iters=5):
    """Steady-state wall-clock ns for one launch of stage1+stage2 with all
    inputs resident on the NeuronCores (no NTFF profiling under axon)."""
    import jax, time

    h = np.asarray(inputs["h"], dtype=np.float32)
    W = np.asarray(inputs["W"], dtype=np.float32)
    bW = np.asarray(inputs["bW"], dtype=np.float32)
    a = np.asarray(inputs["a"], dtype=np.float32)
    ba = np.asarray(inputs["ba"], dtype=np.float32)
    src = np.asarray(inputs["src"], dtype=np.int64)
    dst = np.asarray(inputs["dst"], dtype=np.int64)
    (s1_inputs, blocks, B, lo_w, hi_w, dloc, n_lo_u, n_hi_u, slot_dst,
     iota) = prep_static(src, dst)
    hT, waug, biasb = s1_inputs(h, W, bW, a, ba)
    core_ids = list(range(NCORES))

    if "s1" not in _cache:
        _cache["s1"] = build_stage1()
    nc1 = _cache["s1"]
    ins1 = [{"hT": np.ascontiguousarray(hT[:, c * NSLICE:(c + 1) * NSLICE]),
             "waug": waug, "biasb": biasb} for c in core_ids]
    fn1, args1, onames1 = _make_runner(nc1, ins1)
    o1 = fn1(*args1)
    jax.block_until_ready(o1)
    tcat = np.asarray(o1[onames1.index("tout")])
    nch = NSLICE // 128
    tfull = np.concatenate(
        [tout_to_rows(tcat.reshape(NCORES, 128, nch, TROW)[c])
         for c in range(NCORES)], axis=0)
    sdst = make_sdst(tfull, slot_dst)

    key = ("s2", B, hash((tuple(n_lo_u), tuple(n_hi_u))))
    if key not in _cache:
        _cache[key] = build_stage2(B, n_lo_u, n_hi_u)
    nc2 = _cache[key]
    ins2 = [core_inputs2(tfull, lo_w, hi_w, dloc, sdst, iota, c)
            for c in core_ids]
    fn2, args2, _ = _make_runner(nc2, ins2)
    jax.block_until_ready(fn2(*args2))

    if "noop" not in _cache:
        _cache["noop"] = build_noop()
    ncn = _cache["noop"]
    insn = [{"x": np.zeros((128, 128), np.float32)} for _ in core_ids]
    fnn, argsn, _ = _make_runner(ncn, insn)
    jax.block_until_ready(fnn(*argsn))

    def once(fn, args):
        t0 = time.perf_counter()
        jax.block_until_ready(fn(*args))
        return time.perf_counter() - t0

    R = 4
    if ("s1r", R) not in _cache:
        _cache[("s1r", R)] = build_stage1(repeat=R)
    fn1r, args1r, _ = _make_runner(_cache[("s1r", R)], ins1)
    jax.block_until_ready(fn1r(*args1r))
    if (key, R) not in _cache:
        _cache[(key, R)] = build_stage2(B, n_lo_u, n_hi_u, repeat=R)
    fn2r, args2r, _ = _make_runner(_cache[(key, R)], ins2)
    jax.block_until_ready(fn2r(*args2r))

    t_noop = [once(fnn, argsn) for _ in range(iters)]
    t_s1 = [once(fn1, args1) for _ in range(iters)]
    t_s2 = [once(fn2, args2) for _ in range(iters)]
    t_s1r = [once(fn1r, args1r) for _ in range(iters)]
    t_s2r = [once(fn2r, args2r) for _ in range(iters)]
    f = min(t_noop)
    s1, s2 = min(t_s1), min(t_s2)
    s1r, s2r = min(t_s1r), min(t_s2r)
    e1 = (s1r - s1) / (R - 1)
    e2 = (s2r - s2) / (R - 1)
    print(f"  walls us: noop {f*1e6:.0f}  s1 {s1*1e6:.0f}  s1x{R} {s1r*1e6:.0f}"
          f"  s2 {s2*1e6:.0f}  s2x{R} {s2r*1e6:.0f}")
    print(f"  per-iter device exec: stage1 {e1*1e9:.0f} ns  "
          f"stage2 {e2*1e9:.0f} ns")
    est = max(e1, 0.0) + max(e2, 0.0)
    print(f"  estimated device exec total: {est*1e9:.0f} ns")
    return int(est * 1e9)

